# revision 12
# baseline (speedup 1.0000x reference)
"""LightGCN 2-hop smoothing on 8 Trainium2 NeuronCores.

Strategy (edge-sharded by destination, transfer-minimized):
  - Host: build symmetric directed edge list (2E = 2.5M messages), sort by
    destination, pack into 128-edge chunks grouped by 128-node destination
    blocks. Core c owns destination nodes [c*R, (c+1)*R).
  - The symmetric GCN weight w_e = s[src]*s[dst] (s = deg^-1/2) factorizes,
    so no per-edge weights are shipped: the gather table holds s-prescaled
    rows (t0 = s .* x0 in bf16) and each scatter-add output row is
    postscaled by s[dst] on device. Per-edge metadata is ONE int32 word:
    src_index | dst_slot << 18 (slot 255 marks padding).
  - Each core receives only its 1/8 table shard; the replicated gather
    table is built on device with an AllGather over NeuronLink.
  - Device, per smoothing hop: unpack metadata with DVE bitwise ops, gather
    source rows with one wide indirect DMA per group, build a one-hot
    selection matrix per 128-edge chunk (out[p,f] = (f == slot[p])), and
    matmul-accumulate the chunk's messages into a PSUM tile per destination
    block.
  - x0 is recovered on device from the scaled bf16 shard (x0 = sinv .* t0),
    and the final out = (2*x0 + 2*x1 + x2)/3 is emitted in bf16.
"""

import numpy as np

import concourse.bass as bass
import concourse.bacc as bacc
import concourse.mybir as mybir
import concourse.tile as tile
from concourse.bass import IndirectOffsetOnAxis
from concourse.bass_utils import run_bass_kernel_spmd

NU = 100000          # num users
NI = 100000          # num items
D = 64               # embedding dim
NCORES = 8
NB = 196             # destination blocks per core
GB = 4               # blocks per gather group

F32 = mybir.dt.float32
BF16 = mybir.dt.bfloat16
I32 = mybir.dt.int32
NP_BF16 = mybir.dt.np(mybir.dt.bfloat16)

_PROG_CACHE = {}


def _dims():
    R = NB * 128
    return NU + NI, R, R * NCORES, NB // GB


def _host_prep(u_emb, i_emb, u_idx, i_idx):
    N, R, NPAD, NG = _dims()
    i_g = i_idx.astype(np.int64) + NU
    src = np.concatenate([u_idx.astype(np.int64), i_g])
    dst = np.concatenate([i_g, u_idx.astype(np.int64)])

    deg = np.bincount(src, minlength=NPAD)  # symmetric edges: in-deg == out-deg
    # s = deg^-1/2 where deg>0 else 1; w_e = s[src]*s[dst] (isolated nodes
    # never appear in any edge, so s=1 there is never used as a weight and
    # keeps x0 = sinv*(s*x0) exact for them).
    s = np.where(deg > 0, 1.0 / np.sqrt(np.maximum(deg, 1)), 1.0).astype(np.float32)
    sinv = (1.0 / s).astype(np.float32)

    order = np.argsort(dst, kind="stable")
    src_s = src[order].astype(np.int32)
    dst_s = dst[order]

    nblk_tot = NPAD // 128
    blk = (dst_s >> 7).astype(np.int64)
    nb = np.bincount(blk, minlength=nblk_tot)
    cpb = int(np.ceil(nb.max() / 128))

    starts = np.zeros(nblk_tot, np.int64)
    np.cumsum(nb[:-1], out=starts[1:])
    r = np.arange(len(dst_s)) - starts[blk]
    gchunk = blk * cpb + (r >> 7)
    slot = r & 127

    # one packed word per edge slot: src | dst_slot<<18 (255<<18 = padding)
    nchunks_tot = nblk_tot * cpb
    wordmat = np.full((nchunks_tot, 128), np.int32(255 << 18), np.int32)
    wordmat[gchunk, slot] = src_s | ((dst_s & 127).astype(np.int32) << 18)

    x0 = np.concatenate([np.asarray(u_emb), np.asarray(i_emb)], axis=0)
    t0_pad = np.zeros((NPAD, D), np.float32)
    t0_pad[:N] = x0 * s[:N, None]
    t0_pad = t0_pad.astype(NP_BF16)

    g = GB * cpb  # chunks per gather group
    in_maps = []
    for c in range(NCORES):
        lo, hi = c * NB * cpb, (c + 1) * NB * cpb
        # [nG, 128, G]: element [gi, p, j] belongs to chunk gi*G+j, slot p
        meta = np.ascontiguousarray(
            wordmat[lo:hi].reshape(NG, g, 128).transpose(0, 2, 1))
        rows = slice(c * R, (c + 1) * R)
        sc = np.concatenate([s[rows].reshape(NB, 128).T,
                             sinv[rows].reshape(NB, 128).T], axis=1)
        in_maps.append({
            "t0": np.ascontiguousarray(t0_pad[rows]),
            "meta": meta,
            "sc": np.ascontiguousarray(sc),
        })
    return in_maps, cpb


def _build_program(cpb):
    N, R, NPAD, NG = _dims()
    g = GB * cpb
    nc = bacc.Bacc("TRN2", target_bir_lowering=False, debug=False,
                   num_devices=NCORES)

    t0_in = nc.dram_tensor("t0", [R, D], BF16, kind="ExternalInput").ap()
    meta_in = nc.dram_tensor("meta", [NG, 128, g], I32, kind="ExternalInput").ap()
    sc_in = nc.dram_tensor("sc", [128, 2 * NB], F32, kind="ExternalInput").ap()
    out = nc.dram_tensor("out", [R, D], BF16, kind="ExternalOutput").ap()

    t0i = nc.dram_tensor("t0i", [R, D], BF16).ap()
    x1own_bf = nc.dram_tensor("x1own_bf", [R, D], BF16).ap()
    table0 = nc.dram_tensor("table0", [NPAD, D], BF16, addr_space="Shared").ap()
    table1 = nc.dram_tensor("table1", [NPAD, D], BF16, addr_space="Shared").ap()

    with tile.TileContext(nc) as tc:
        with (
            tc.tile_pool(name="persist", bufs=1) as persist,
            tc.tile_pool(name="meta", bufs=3) as mp,
            tc.tile_pool(name="gather", bufs=3) as gp,
            tc.tile_pool(name="oh", bufs=8) as ohp,
            tc.tile_pool(name="ev", bufs=3) as ev,
            tc.tile_pool(name="psum", bufs=8, space="PSUM") as pp,
        ):
            sc_t = persist.tile([128, 2 * NB], F32)
            nc.sync.dma_start(out=sc_t[:], in_=sc_in[:])
            iota_i = persist.tile([128, 128], I32)
            nc.gpsimd.iota(out=iota_i[:], pattern=[[1, 128]], base=0,
                           channel_multiplier=0)
            iota_t = persist.tile([128, 128], F32)
            nc.scalar.activation(out=iota_t[:], in_=iota_i[:],
                                 func=mybir.ActivationFunctionType.Copy)
            # fp32 copy of this core's x1 shard, kept in SBUF between hops
            x1keep = persist.tile([128, NB * D], F32)

            # replicate the scaled-x0 shards into the gather table
            # (collectives cannot read IO tensors; stage through t0i)
            nc.sync.dma_start(out=t0i[:], in_=t0_in[:])
            nc.gpsimd.collective_compute(
                "AllGather", mybir.AluOpType.bypass,
                replica_groups=[list(range(NCORES))],
                ins=[t0i[:]], outs=[table0[:]],
            )

            def smooth(hop, table_ap):
                for gi in range(NG):
                    meta_t = mp.tile([128, g], I32, tag="meta")
                    nc.sync.dma_start(out=meta_t[:], in_=meta_in[gi])
                    srcx = mp.tile([128, g], I32, tag="srcx")
                    nc.vector.tensor_scalar(
                        out=srcx[:], in0=meta_t[:], scalar1=0x3FFFF,
                        scalar2=None, op0=mybir.AluOpType.bitwise_and)
                    sloti = mp.tile([128, g], I32, tag="sloti")
                    nc.vector.tensor_scalar(
                        out=sloti[:], in0=meta_t[:], scalar1=18,
                        scalar2=None, op0=mybir.AluOpType.logical_shift_right)
                    slotf = mp.tile([128, g], F32, tag="slotf")
                    nc.scalar.activation(
                        out=slotf[:], in_=sloti[:],
                        func=mybir.ActivationFunctionType.Copy)

                    gbuf = gp.tile([128, g * D], BF16, tag="gbuf")
                    # HW indirect DMA consumes one index per dest partition
                    # row, so gather 128 rows per call.
                    for j in range(g):
                        nc.gpsimd.indirect_dma_start(
                            out=gbuf[:, j * D:(j + 1) * D], out_offset=None,
                            in_=table_ap,
                            in_offset=IndirectOffsetOnAxis(
                                ap=srcx[:, j:j + 1], axis=0),
                        )

                    for jb in range(GB):
                        b = gi * GB + jb
                        psum = pp.tile([128, D], F32, tag="psum")
                        for k in range(cpb):
                            j = jb * cpb + k
                            oh = ohp.tile([128, 128], BF16, tag="oh")
                            nc.vector.tensor_scalar(
                                out=oh[:], in0=iota_t[:],
                                scalar1=slotf[:, j:j + 1], scalar2=None,
                                op0=mybir.AluOpType.is_equal,
                            )
                            nc.tensor.matmul(
                                out=psum[:], lhsT=oh[:],
                                rhs=gbuf[:, j * D:(j + 1) * D],
                                start=(k == 0), stop=(k == cpb - 1),
                            )
                        rows = slice(b * 128, (b + 1) * 128)
                        if hop == 0:
                            # x1 = s * psum (keep f32); table1 row = s * x1
                            nc.vector.tensor_scalar(
                                out=x1keep[:, b * D:(b + 1) * D], in0=psum[:],
                                scalar1=sc_t[:, b:b + 1], scalar2=None,
                                op0=mybir.AluOpType.mult)
                            x1s = ev.tile([128, D], BF16, tag="x1s")
                            nc.scalar.activation(
                                out=x1s[:], in_=x1keep[:, b * D:(b + 1) * D],
                                func=mybir.ActivationFunctionType.Copy,
                                scale=sc_t[:, b:b + 1])
                            nc.sync.dma_start(out=x1own_bf[rows], in_=x1s[:])
                        else:
                            t0blk = ev.tile([128, D], BF16, tag="t0blk")
                            nc.sync.dma_start(out=t0blk[:], in_=t0_in[rows])
                            x0f = ev.tile([128, D], F32, tag="x0f")
                            nc.scalar.activation(
                                out=x0f[:], in_=t0blk[:],
                                func=mybir.ActivationFunctionType.Copy,
                                scale=sc_t[:, NB + b:NB + b + 1])
                            a01 = ev.tile([128, D], F32, tag="a01")
                            nc.vector.tensor_tensor(
                                out=a01[:], in0=x0f[:],
                                in1=x1keep[:, b * D:(b + 1) * D],
                                op=mybir.AluOpType.add)
                            x2t = ev.tile([128, D], F32, tag="x2t")
                            nc.scalar.activation(
                                out=x2t[:], in_=psum[:],
                                func=mybir.ActivationFunctionType.Copy,
                                scale=sc_t[:, b:b + 1])
                            a2 = ev.tile([128, D], F32, tag="a2")
                            nc.vector.tensor_scalar(
                                out=a2[:], in0=a01[:], scalar1=2.0,
                                scalar2=None, op0=mybir.AluOpType.mult)
                            u = ev.tile([128, D], F32, tag="u")
                            nc.vector.tensor_tensor(
                                out=u[:], in0=a2[:], in1=x2t[:],
                                op=mybir.AluOpType.add)
                            obuf = ev.tile([128, D], BF16, tag="obuf")
                            nc.scalar.activation(
                                out=obuf[:], in_=u[:],
                                func=mybir.ActivationFunctionType.Copy,
                                scale=1.0 / 3.0)
                            nc.sync.dma_start(out=out[rows], in_=obuf[:])

            smooth(0, table0[:])
            nc.gpsimd.collective_compute(
                "AllGather", mybir.AluOpType.bypass,
                replica_groups=[list(range(NCORES))],
                ins=[x1own_bf[:]], outs=[table1[:]],
            )
            smooth(1, table1[:])

    nc.compile()
    return nc


def _get_program(cpb):
    if cpb not in _PROG_CACHE:
        _PROG_CACHE[cpb] = _build_program(cpb)
    return _PROG_CACHE[cpb]


def kernel(u_emb, i_emb, u_idx, i_idx):
    N, R, NPAD, NG = _dims()
    in_maps, cpb = _host_prep(u_emb, i_emb, u_idx, i_idx)
    nc = _get_program(cpb)
    res = run_bass_kernel_spmd(nc, in_maps, list(range(NCORES)))
    full = np.concatenate([res.results[c]["out"] for c in range(NCORES)], axis=0)
    return np.ascontiguousarray(full[:N]).astype(np.float32)


# revision 23
# speedup vs baseline: 1.3468x; 1.3468x over previous
"""LightGCN 2-hop smoothing on 8 Trainium2 NeuronCores.

Strategy (edge-sharded by destination, transfer-minimized):
  - Host: build symmetric directed edge list (2E = 2.5M messages), sort by
    destination, pack into 128-edge chunks grouped by 128-node destination
    blocks. Core c owns destination nodes [c*R, (c+1)*R).
  - The symmetric GCN weight w_e = s[src]*s[dst] (s = deg^-1/2) factorizes,
    so no per-edge weights are shipped: the gather table holds s-prescaled
    rows (t0 = s .* x0 in bf16) and each scatter-add output row is
    postscaled by s[dst] on device. Per-edge metadata is ONE int32 word:
    src_index | dst_slot << 18 (slot 255 marks padding).
  - Each core receives only its 1/8 table shard; the replicated gather
    table is built on device with an AllGather over NeuronLink.
  - Device, per smoothing hop: unpack metadata with DVE bitwise ops, gather
    source rows with one wide indirect DMA per group, build a one-hot
    selection matrix per 128-edge chunk (out[p,f] = (f == slot[p])), and
    matmul-accumulate the chunk's messages into a PSUM tile per destination
    block.
  - The gather table is fp8(e4m3) with a x16 prescale folded into the
    per-row scales (t0 = 16*s .* x0); gathered rows are upconverted to bf16
    on device before the matmul. The device emits only the smoothed delta
    16*(2*x1 + x2)/3 in fp8; the host adds the exact (2/3)*x0 term in f32,
    so fp8 quantization only touches the small smoothed component.
"""

import numpy as np

import concourse.bass as bass
import concourse.bacc as bacc
import concourse.mybir as mybir
import concourse.tile as tile
from concourse.bass import IndirectOffsetOnAxis
from concourse.bass_utils import run_bass_kernel_spmd

NU = 100000          # num users
NI = 100000          # num items
D = 64               # embedding dim
NCORES = 8
NB = 196             # destination blocks per core
GB = 4               # blocks per gather group

F32 = mybir.dt.float32
BF16 = mybir.dt.bfloat16
I32 = mybir.dt.int32
F8 = mybir.dt.float8e4
NP_F8 = mybir.dt.np(mybir.dt.float8e4)
P = 16.0             # fp8 prescale, folded into per-row scales

_PROG_CACHE = {}


def _dims():
    R = NB * 128
    return NU + NI, R, R * NCORES, NB // GB


def _host_prep(u_emb, i_emb, u_idx, i_idx):
    N, R, NPAD, NG = _dims()
    i_g = i_idx.astype(np.int64) + NU
    src = np.concatenate([u_idx.astype(np.int64), i_g])
    dst = np.concatenate([i_g, u_idx.astype(np.int64)])

    deg = np.bincount(src, minlength=NPAD)  # symmetric edges: in-deg == out-deg
    # s = deg^-1/2 where deg>0 else 1; w_e = s[src]*s[dst] (isolated nodes
    # never appear in any edge, so s=1 there is never used as a weight and
    # keeps x0 = sinv*(s*x0) exact for them).
    s = np.where(deg > 0, 1.0 / np.sqrt(np.maximum(deg, 1)), 1.0).astype(np.float32)

    order = np.argsort(dst, kind="stable")
    src_s = src[order].astype(np.int32)
    dst_s = dst[order]

    nblk_tot = NPAD // 128
    blk = (dst_s >> 7).astype(np.int64)
    nb = np.bincount(blk, minlength=nblk_tot)
    cpb = int(np.ceil(nb.max() / 128))

    starts = np.zeros(nblk_tot, np.int64)
    np.cumsum(nb[:-1], out=starts[1:])
    r = np.arange(len(dst_s)) - starts[blk]
    gchunk = blk * cpb + (r >> 7)
    slot = r & 127

    # one packed word per edge slot: src | dst_slot<<18 (255<<18 = padding)
    nchunks_tot = nblk_tot * cpb
    wordmat = np.full((nchunks_tot, 128), np.int32(255 << 18), np.int32)
    wordmat[gchunk, slot] = src_s | ((dst_s & 127).astype(np.int32) << 18)

    x0 = np.concatenate([np.asarray(u_emb), np.asarray(i_emb)], axis=0)
    t0_pad = np.zeros((NPAD, D), np.float32)
    t0_pad[:N] = x0 * (P * s[:N, None])
    t0_pad = t0_pad.astype(NP_F8)

    g = GB * cpb  # chunks per gather group
    in_maps = []
    for c in range(NCORES):
        lo, hi = c * NB * cpb, (c + 1) * NB * cpb
        # [nG, 128, G]: element [gi, p, j] belongs to chunk gi*G+j, slot p
        meta = np.ascontiguousarray(
            wordmat[lo:hi].reshape(NG, g, 128).transpose(0, 2, 1))
        rows = slice(c * R, (c + 1) * R)
        s2d = s[rows].reshape(NB, 128).T
        sc = np.concatenate([s2d / P, P * s2d], axis=1)
        in_maps.append({
            "t0": np.ascontiguousarray(t0_pad[rows]),
            "meta": meta,
            "sc": np.ascontiguousarray(sc),
        })
    return in_maps, cpb


def _build_program(cpb):
    N, R, NPAD, NG = _dims()
    g = GB * cpb
    nc = bacc.Bacc("TRN2", target_bir_lowering=False, debug=False,
                   num_devices=NCORES)

    t0_in = nc.dram_tensor("t0", [R, D], F8, kind="ExternalInput").ap()
    meta_in = nc.dram_tensor("meta", [NG, 128, g], I32, kind="ExternalInput").ap()
    sc_in = nc.dram_tensor("sc", [128, 2 * NB], F32, kind="ExternalInput").ap()
    out = nc.dram_tensor("out", [R, D], F8, kind="ExternalOutput").ap()

    t0i = nc.dram_tensor("t0i", [R, D], F8).ap()
    x1own_f8 = nc.dram_tensor("x1own_f8", [R, D], F8).ap()
    table0 = nc.dram_tensor("table0", [NPAD, D], F8, addr_space="Shared").ap()
    table1 = nc.dram_tensor("table1", [NPAD, D], F8, addr_space="Shared").ap()

    with tile.TileContext(nc) as tc:
        with (
            tc.tile_pool(name="persist", bufs=1) as persist,
            tc.tile_pool(name="meta", bufs=3) as mp,
            tc.tile_pool(name="gather", bufs=3) as gp,
            tc.tile_pool(name="oh", bufs=8) as ohp,
            tc.tile_pool(name="ev", bufs=3) as ev,
            tc.tile_pool(name="psum", bufs=8, space="PSUM") as pp,
        ):
            sc_t = persist.tile([128, 2 * NB], F32)
            nc.sync.dma_start(out=sc_t[:], in_=sc_in[:])
            iota_i = persist.tile([128, 128], I32)
            nc.gpsimd.iota(out=iota_i[:], pattern=[[1, 128]], base=0,
                           channel_multiplier=0)
            iota_t = persist.tile([128, 128], F32)
            nc.scalar.activation(out=iota_t[:], in_=iota_i[:],
                                 func=mybir.ActivationFunctionType.Copy)
            # fp32 copy of this core's x1 shard, kept in SBUF between hops
            x1keep = persist.tile([128, NB * D], F32)

            # replicate the scaled-x0 shards into the gather table
            # (collectives cannot read IO tensors; stage through t0i)
            nc.sync.dma_start(out=t0i[:], in_=t0_in[:])
            nc.gpsimd.collective_compute(
                "AllGather", mybir.AluOpType.bypass,
                replica_groups=[list(range(NCORES))],
                ins=[t0i[:]], outs=[table0[:]],
            )

            def smooth(hop, table_ap):
                for gi in range(NG):
                    meta_t = mp.tile([128, g], I32, tag="meta")
                    nc.sync.dma_start(out=meta_t[:], in_=meta_in[gi])
                    srcx = mp.tile([128, g], I32, tag="srcx")
                    nc.vector.tensor_scalar(
                        out=srcx[:], in0=meta_t[:], scalar1=0x3FFFF,
                        scalar2=None, op0=mybir.AluOpType.bitwise_and)
                    sloti = mp.tile([128, g], I32, tag="sloti")
                    nc.vector.tensor_scalar(
                        out=sloti[:], in0=meta_t[:], scalar1=18,
                        scalar2=None, op0=mybir.AluOpType.logical_shift_right)
                    slotf = mp.tile([128, g], F32, tag="slotf")
                    nc.scalar.activation(
                        out=slotf[:], in_=sloti[:],
                        func=mybir.ActivationFunctionType.Copy)

                    gbuf = gp.tile([128, g * D], F8, tag="gbuf")
                    # HW indirect DMA consumes one index per dest partition
                    # row, so gather 128 rows per call.
                    for j in range(g):
                        nc.gpsimd.indirect_dma_start(
                            out=gbuf[:, j * D:(j + 1) * D], out_offset=None,
                            in_=table_ap,
                            in_offset=IndirectOffsetOnAxis(
                                ap=srcx[:, j:j + 1], axis=0),
                        )
                    gbh = gp.tile([128, g * D], BF16, tag="gbh")
                    nc.scalar.activation(
                        out=gbh[:], in_=gbuf[:],
                        func=mybir.ActivationFunctionType.Copy)

                    for jb in range(GB):
                        b = gi * GB + jb
                        psum = pp.tile([128, D], F32, tag="psum")
                        for k in range(cpb):
                            j = jb * cpb + k
                            oh = ohp.tile([128, 128], BF16, tag="oh")
                            nc.vector.tensor_scalar(
                                out=oh[:], in0=iota_t[:],
                                scalar1=slotf[:, j:j + 1], scalar2=None,
                                op0=mybir.AluOpType.is_equal,
                            )
                            nc.tensor.matmul(
                                out=psum[:], lhsT=oh[:],
                                rhs=gbh[:, j * D:(j + 1) * D],
                                start=(k == 0), stop=(k == cpb - 1),
                            )
                        rows = slice(b * 128, (b + 1) * 128)
                        if hop == 0:
                            # x1 = (s/P) * psum (keep f32); table1 = P*s*x1
                            nc.vector.tensor_scalar(
                                out=x1keep[:, b * D:(b + 1) * D], in0=psum[:],
                                scalar1=sc_t[:, b:b + 1], scalar2=None,
                                op0=mybir.AluOpType.mult)
                            x1s = ev.tile([128, D], F8, tag="x1s")
                            nc.scalar.activation(
                                out=x1s[:], in_=x1keep[:, b * D:(b + 1) * D],
                                func=mybir.ActivationFunctionType.Copy,
                                scale=sc_t[:, NB + b:NB + b + 1])
                            nc.sync.dma_start(out=x1own_f8[rows], in_=x1s[:])
                        else:
                            # delta' = P*(2*x1 + x2)/3, x2 = (s/P)*psum
                            x2t = ev.tile([128, D], F32, tag="x2t")
                            nc.scalar.activation(
                                out=x2t[:], in_=psum[:],
                                func=mybir.ActivationFunctionType.Copy,
                                scale=sc_t[:, b:b + 1])
                            a2 = ev.tile([128, D], F32, tag="a2")
                            nc.vector.tensor_scalar(
                                out=a2[:], in0=x1keep[:, b * D:(b + 1) * D],
                                scalar1=2.0, scalar2=None,
                                op0=mybir.AluOpType.mult)
                            u = ev.tile([128, D], F32, tag="u")
                            nc.vector.tensor_tensor(
                                out=u[:], in0=a2[:], in1=x2t[:],
                                op=mybir.AluOpType.add)
                            obuf = ev.tile([128, D], F8, tag="obuf")
                            nc.scalar.activation(
                                out=obuf[:], in_=u[:],
                                func=mybir.ActivationFunctionType.Copy,
                                scale=P / 3.0)
                            nc.sync.dma_start(out=out[rows], in_=obuf[:])

            smooth(0, table0[:])
            nc.gpsimd.collective_compute(
                "AllGather", mybir.AluOpType.bypass,
                replica_groups=[list(range(NCORES))],
                ins=[x1own_f8[:]], outs=[table1[:]],
            )
            smooth(1, table1[:])

    nc.compile()
    return nc


def _get_program(cpb):
    if cpb not in _PROG_CACHE:
        _PROG_CACHE[cpb] = _build_program(cpb)
    return _PROG_CACHE[cpb]


def _assemble(u_emb, i_emb, delta_full):
    N, R, NPAD, NG = _dims()
    x0 = np.concatenate([np.asarray(u_emb), np.asarray(i_emb)], axis=0)
    delta = delta_full[:N].astype(np.float32)
    return (2.0 / 3.0) * x0 + delta * (1.0 / P)


def kernel(u_emb, i_emb, u_idx, i_idx):
    in_maps, cpb = _host_prep(u_emb, i_emb, u_idx, i_idx)
    nc = _get_program(cpb)
    res = run_bass_kernel_spmd(nc, in_maps, list(range(NCORES)))
    full = np.concatenate([res.results[c]["out"] for c in range(NCORES)], axis=0)
    return np.ascontiguousarray(_assemble(u_emb, i_emb, full)).astype(np.float32)


# revision 30
# speedup vs baseline: 1.7116x; 1.2708x over previous
"""LightGCN 2-hop smoothing on 8 Trainium2 NeuronCores.

Strategy (edge-sharded by destination, transfer-minimized):
  - Host: build symmetric directed edge list (2E = 2.5M messages), sort by
    destination, pack into 128-edge chunks grouped by 128-node destination
    blocks. Core c owns destination nodes [c*R, (c+1)*R).
  - The symmetric GCN weight w_e = s[src]*s[dst] (s = deg^-1/2) factorizes,
    so no per-edge weights are shipped: the gather table holds s-prescaled
    rows (t0 = s .* x0 in bf16) and each scatter-add output row is
    postscaled by s[dst] on device. Per-edge metadata is ONE int32 word:
    src_index | dst_slot << 18 (slot 255 marks padding).
  - Each core receives only its 1/8 table shard; the replicated gather
    table is built on device with an AllGather over NeuronLink.
  - Device, per smoothing hop: unpack metadata with DVE bitwise ops, gather
    source rows with one wide indirect DMA per group, build a one-hot
    selection matrix per 128-edge chunk (out[p,f] = (f == slot[p])), and
    matmul-accumulate the chunk's messages into a PSUM tile per destination
    block.
  - The gather table is fp8(e4m3) with a x16 prescale folded into the
    per-row scales (t0 = 16*s .* x0); gathered rows are upconverted to bf16
    on device before the matmul. The device emits only the smoothed delta
    16*(2*x1 + x2)/3 in fp8; the host adds the exact (2/3)*x0 term in f32,
    so fp8 quantization only touches the small smoothed component.
"""

import numpy as np

import concourse.bass as bass
import concourse.bacc as bacc
import concourse.mybir as mybir
import concourse.tile as tile
from concourse.bass import IndirectOffsetOnAxis
from concourse.bass_utils import run_bass_kernel_spmd

NU = 100000          # num users
NI = 100000          # num items
D = 64               # embedding dim
NCORES = 8
NB = 196             # destination blocks per core
GB = 4               # blocks per gather group

F32 = mybir.dt.float32
BF16 = mybir.dt.bfloat16
I32 = mybir.dt.int32
F8 = mybir.dt.float8e4
NP_F8 = mybir.dt.np(mybir.dt.float8e4)
NP_BF16 = mybir.dt.np(mybir.dt.bfloat16)
P = 16.0             # fp8 prescale, folded into per-row scales

_PROG_CACHE = {}


def _dims():
    R = NB * 128
    return NU + NI, R, R * NCORES, NB // GB


def _host_prep(u_emb, i_emb, u_idx, i_idx):
    N, R, NPAD, NG = _dims()
    i_g = i_idx.astype(np.int64) + NU
    src = np.concatenate([u_idx.astype(np.int64), i_g])
    dst = np.concatenate([i_g, u_idx.astype(np.int64)])

    deg = np.bincount(src, minlength=NPAD)  # symmetric edges: in-deg == out-deg
    # s = deg^-1/2 where deg>0 else 1; w_e = s[src]*s[dst] (isolated nodes
    # never appear in any edge, so s=1 there is never used as a weight and
    # keeps x0 = sinv*(s*x0) exact for them).
    s = np.where(deg > 0, 1.0 / np.sqrt(np.maximum(deg, 1)), 1.0).astype(np.float32)

    order = np.argsort(dst, kind="stable")
    src_s = src[order].astype(np.int32)
    dst_s = dst[order]

    nblk_tot = NPAD // 128
    blk = (dst_s >> 7).astype(np.int64)
    nb = np.bincount(blk, minlength=nblk_tot)
    cpb = int(np.ceil(nb.max() / 128))

    starts = np.zeros(nblk_tot, np.int64)
    np.cumsum(nb[:-1], out=starts[1:])
    r = np.arange(len(dst_s)) - starts[blk]
    gchunk = blk * cpb + (r >> 7)
    slot = r & 127

    # one packed word per edge slot: src | dst_slot<<18 (255<<18 = padding)
    nchunks_tot = nblk_tot * cpb
    wordmat = np.full((nchunks_tot, 128), np.int32(255 << 18), np.int32)
    wordmat[gchunk, slot] = src_s | ((dst_s & 127).astype(np.int32) << 18)

    x0 = np.concatenate([np.asarray(u_emb), np.asarray(i_emb)], axis=0)
    t0_pad = np.zeros((NPAD, D), np.float32)
    t0_pad[:N] = x0 * (P * s[:N, None])
    t0_pad = t0_pad.astype(NP_F8)

    g = GB * cpb  # chunks per gather group
    in_maps = []
    for c in range(NCORES):
        lo, hi = c * NB * cpb, (c + 1) * NB * cpb
        # [nG, 128, G]: element [gi, p, j] belongs to chunk gi*G+j, slot p
        meta = np.ascontiguousarray(
            wordmat[lo:hi].reshape(NG, g, 128).transpose(0, 2, 1))
        rows = slice(c * R, (c + 1) * R)
        s2d = s[rows].reshape(NB, 128).T
        # col set A: s^2 (x1own = s^2 * psum1); col set B: s/3 (out scale)
        sc = np.concatenate([s2d * s2d, s2d / 3.0], axis=1)
        in_maps.append({
            "t0": np.ascontiguousarray(t0_pad[rows]),
            "meta": meta,
            "sc": np.ascontiguousarray(sc),
            "d2": (2.0 * np.eye(128, dtype=np.float32)).astype(NP_BF16),
        })
    return in_maps, cpb


def _build_program(cpb):
    N, R, NPAD, NG = _dims()
    g = GB * cpb
    nc = bacc.Bacc("TRN2", target_bir_lowering=False, debug=False,
                   num_devices=NCORES)

    t0_in = nc.dram_tensor("t0", [R, D], F8, kind="ExternalInput").ap()
    meta_in = nc.dram_tensor("meta", [NG, 128, g], I32, kind="ExternalInput").ap()
    sc_in = nc.dram_tensor("sc", [128, 2 * NB], F32, kind="ExternalInput").ap()
    d2_in = nc.dram_tensor("d2", [128, 128], BF16, kind="ExternalInput").ap()
    out = nc.dram_tensor("out", [R, D], F8, kind="ExternalOutput").ap()

    t0i = nc.dram_tensor("t0i", [R, D], F8).ap()
    x1own_f8 = nc.dram_tensor("x1own_f8", [R, D], F8).ap()
    table0 = nc.dram_tensor("table0", [NPAD, D], F8, addr_space="Shared").ap()
    table1 = nc.dram_tensor("table1", [NPAD, D], F8, addr_space="Shared").ap()

    with tile.TileContext(nc) as tc:
        with (
            tc.tile_pool(name="persist", bufs=1) as persist,
            tc.tile_pool(name="gather", bufs=3) as gp,
            tc.tile_pool(name="oh", bufs=4) as ohp,
            tc.tile_pool(name="ev", bufs=3) as ev,
            tc.tile_pool(name="psum", bufs=8, space="PSUM") as pp,
        ):
            sc_t = persist.tile([128, 2 * NB], F32)
            nc.sync.dma_start(out=sc_t[:], in_=sc_in[:])
            d2_t = persist.tile([128, 128], BF16)
            nc.sync.dma_start(out=d2_t[:], in_=d2_in[:])
            iota_i = persist.tile([128, 128], I32)
            nc.gpsimd.iota(out=iota_i[:], pattern=[[1, 128]], base=0,
                           channel_multiplier=0)
            iota_t = persist.tile([128, 128], F32)
            nc.scalar.activation(out=iota_t[:], in_=iota_i[:],
                                 func=mybir.ActivationFunctionType.Copy)
            # raw hop-1 PSUM sums (2E-weighted), kept in SBUF between hops
            x1keep = persist.tile([128, NB * D], BF16)

            # load + unpack ALL edge metadata once (shared by both hops)
            meta_t = persist.tile([128, NG * g], I32)
            for gi in range(NG):
                nc.sync.dma_start(
                    out=meta_t[:, gi * g:(gi + 1) * g], in_=meta_in[gi])
            srcx = persist.tile([128, NG * g], I32)
            nc.vector.tensor_scalar(
                out=srcx[:], in0=meta_t[:], scalar1=0x3FFFF,
                scalar2=None, op0=mybir.AluOpType.bitwise_and)
            sloti = persist.tile([128, NG * g], I32)
            nc.vector.tensor_scalar(
                out=sloti[:], in0=meta_t[:], scalar1=18,
                scalar2=None, op0=mybir.AluOpType.logical_shift_right)
            slotf = persist.tile([128, NG * g], F32)
            nc.scalar.activation(
                out=slotf[:], in_=sloti[:],
                func=mybir.ActivationFunctionType.Copy)

            # replicate the scaled-x0 shards into the gather table
            # (collectives cannot read IO tensors; stage through t0i)
            nc.sync.dma_start(out=t0i[:], in_=t0_in[:])
            nc.gpsimd.collective_compute(
                "AllGather", mybir.AluOpType.bypass,
                replica_groups=[list(range(NCORES))],
                ins=[t0i[:]], outs=[table0[:]],
            )

            def smooth(hop, table_ap):
                for gi in range(NG):
                    gbuf = gp.tile([128, g * D], F8, tag="gbuf")
                    # HW indirect DMA consumes one index per dest partition
                    # row, so gather 128 rows per call.
                    for j in range(g):
                        jj = gi * g + j
                        nc.gpsimd.indirect_dma_start(
                            out=gbuf[:, j * D:(j + 1) * D], out_offset=None,
                            in_=table_ap,
                            in_offset=IndirectOffsetOnAxis(
                                ap=srcx[:, jj:jj + 1], axis=0),
                        )
                    gbh = gp.tile([128, g * D], BF16, tag="gbh")
                    nc.scalar.activation(
                        out=gbh[:], in_=gbuf[:],
                        func=mybir.ActivationFunctionType.Copy)

                    for jb in range(GB):
                        b = gi * GB + jb
                        j0 = gi * g + jb * cpb
                        # all cpb one-hots for this block in ONE DVE op
                        oh = ohp.tile([128, cpb * 128], BF16, tag="oh")
                        nc.vector.tensor_tensor(
                            out=oh[:].rearrange("p (k f) -> p k f", f=128),
                            in0=iota_t[:].unsqueeze(1).to_broadcast(
                                [128, cpb, 128]),
                            in1=slotf[:, j0:j0 + cpb].unsqueeze(2).to_broadcast(
                                [128, cpb, 128]),
                            op=mybir.AluOpType.is_equal)
                        psum = pp.tile([128, D], F32, tag="psum")
                        for k in range(cpb):
                            j = jb * cpb + k
                            nc.tensor.matmul(
                                out=psum[:], lhsT=oh[:, k * 128:(k + 1) * 128],
                                rhs=gbh[:, j * D:(j + 1) * D],
                                start=(k == 0),
                                stop=(k == cpb - 1) and hop == 0,
                            )
                        rows = slice(b * 128, (b + 1) * 128)
                        if hop == 0:
                            # keep raw hop-1 sums; x1own = s^2 * psum1
                            nc.scalar.activation(
                                out=x1keep[:, b * D:(b + 1) * D], in_=psum[:],
                                func=mybir.ActivationFunctionType.Copy)
                            x1s = ev.tile([128, D], F8, tag="x1s")
                            nc.scalar.activation(
                                out=x1s[:], in_=psum[:],
                                func=mybir.ActivationFunctionType.Copy,
                                scale=sc_t[:, b:b + 1])
                            nc.sync.dma_start(out=x1own_f8[rows], in_=x1s[:])
                        else:
                            # psum += 2I @ x1keep  ->  delta' = (s/3)*psum
                            nc.tensor.matmul(
                                out=psum[:], lhsT=d2_t[:],
                                rhs=x1keep[:, b * D:(b + 1) * D],
                                start=False, stop=True,
                            )
                            obuf = ev.tile([128, D], F8, tag="obuf")
                            nc.scalar.activation(
                                out=obuf[:], in_=psum[:],
                                func=mybir.ActivationFunctionType.Copy,
                                scale=sc_t[:, NB + b:NB + b + 1])
                            nc.sync.dma_start(out=out[rows], in_=obuf[:])

            smooth(0, table0[:])
            nc.gpsimd.collective_compute(
                "AllGather", mybir.AluOpType.bypass,
                replica_groups=[list(range(NCORES))],
                ins=[x1own_f8[:]], outs=[table1[:]],
            )
            smooth(1, table1[:])

    nc.compile()
    return nc


def _get_program(cpb):
    if cpb not in _PROG_CACHE:
        _PROG_CACHE[cpb] = _build_program(cpb)
    return _PROG_CACHE[cpb]


def _assemble(u_emb, i_emb, delta_full):
    N, R, NPAD, NG = _dims()
    x0 = np.concatenate([np.asarray(u_emb), np.asarray(i_emb)], axis=0)
    delta = delta_full[:N].astype(np.float32)
    return (2.0 / 3.0) * x0 + delta * (1.0 / P)


def kernel(u_emb, i_emb, u_idx, i_idx):
    in_maps, cpb = _host_prep(u_emb, i_emb, u_idx, i_idx)
    nc = _get_program(cpb)
    res = run_bass_kernel_spmd(nc, in_maps, list(range(NCORES)))
    full = np.concatenate([res.results[c]["out"] for c in range(NCORES)], axis=0)
    return np.ascontiguousarray(_assemble(u_emb, i_emb, full)).astype(np.float32)
